# revision 55
# baseline (speedup 1.0000x reference)
"""Causal self-attention with interleaved RoPE on 8 NeuronCores.

Sharding: batch x tensor-parallel. Core c handles batch c//4 and heads
4*(c%4) .. 4*(c%4)+3 (two head-pairs hp=0,1). Each core loads only its
batch's activations (bf16), computes QKV + RoPE + attention for its 4
heads, and writes a bf16 partial output [1024, T] (contraction over its
256 head dims); the host sums 4 partials per batch and adds the bias.

Per-core structure (per head-pair hp, packed heads hA, hB):
  - On-chip tensors live "transposed": feature dim on partitions, tokens
    on the free dim. Input DMAs are spread across the sync/scalar/gpsimd
    queues (each DMA_DIRECT2D issue costs ~1us of sequencer time); the x
    stream owns sync and late-needed weights are issued behind quarter-0
    compute so x keeps the HBM bandwidth through the fill.
  - QKV q,k: psum[row, tok] = w_tile.T @ x_tile (contraction over C in 8
    bf16 tiles). RoPE applied in-transposed layout via DVE stream_shuffle
    with a sign-folded, pair-reindexed sin table.
  - V is produced directly token-major: psum[tok, dim] = x_blk.T @ wv_tile
    (stationary = x block, moving = v weights); one copy lands it in the
    persistent AV stationary tile [V_A | V_B | ones] whose ones block
    makes the AV matmul also emit softmax row sums.
  - Scores transposed: S^T[tk, tq] = K^T.T @ Q^T per head, 2 heads packed
    via PE row tiling. Causal masking via subrange matmuls/exp on diagonal
    tiles plus a host tri mask for the partial block. exp on ACT (scale
    1/8 + key-mask bias folded in).
  - Softmax normalize uses the 1-instruction DVE approx reciprocal (staged
    through SBUF; its bit-trick seed misreads PSUM) so the yp PSUM buffers
    recycle in ~2us instead of 8us of exact-reciprocal latency.
  - Schedule: QKV(0) quarter 0; attn(0) starts immediately, fed by the
    remaining QKV(0) quarters + QKV(1) quarters 0-1 (gated per block);
    attn(1) is fed by QKV(1) quarters 2-3 first (its out-proj units only
    appear after the first normalize) and then per-block out-proj units,
    with the last block's rate lowered so leftover units keep the PE (and
    the HAM clock) warm through the final normalize window.
  - Out-projection: per query block, 8 units of 2 accumulating bf16
    matmuls (contraction 256 over both hps) + DVE bf16 copy; bf16 partials
    leave via gpsimd SWDGE in per-2mt pieces so the final drain never sits
    on a large transfer.
"""

import numpy as np

B, T, C = 2, 2048, 1024
H, DH = 16, 64
NCORES = 8
CT = C // 128  # 8 contraction tiles
NTK = T // 128  # 16 key tiles
NJ = T // 512  # 4 query blocks
NEG = -1e30

_PROGRAM_CACHE = {}
LAST_RESULTS = None


def _build_program(has_qkv_bias=False):
    import concourse.mybir as mybir
    import concourse.tile as tile
    from concourse import bacc
    from contextlib import ExitStack

    F32 = mybir.dt.float32
    F32R = mybir.dt.float32r
    BF16 = mybir.dt.bfloat16
    EXP = mybir.ActivationFunctionType.Exp
    LN = mybir.ActivationFunctionType.Ln

    SWAP_MASK = [i ^ 1 for i in range(32)]
    nc = bacc.Bacc("TRN2", target_bir_lowering=False, debug=False)

    # ---- DRAM I/O ----
    xT_d = nc.dram_tensor("xT", (NJ, 128, CT, 512), BF16, kind="ExternalInput")
    qkvwT_d = nc.dram_tensor("qkvwT", (2, 3, 128, CT, 128), BF16, kind="ExternalInput")
    bqkv_d = nc.dram_tensor("bqkv", (2, 128, 4), F32, kind="ExternalInput")
    vbb_d = nc.dram_tensor("vbb", (2, 128, 128), F32, kind="ExternalInput")
    owT_d = nc.dram_tensor("owT", (2, 128, 8, 128), BF16, kind="ExternalInput")
    cosT_d = nc.dram_tensor("cosT", (128, T), BF16, kind="ExternalInput")
    sinTt_d = nc.dram_tensor("sinTt", (128, T), BF16, kind="ExternalInput")
    triC_d = nc.dram_tensor("triC", (128, 128), BF16, kind="ExternalInput")
    expb_d = nc.dram_tensor("expb", (128, NTK), F32, kind="ExternalInput")
    outp_d = nc.dram_tensor("outp", (128, 8, T), BF16, kind="ExternalOutput")

    with tile.TileContext(nc) as tc, ExitStack() as ctx:
        cpool = ctx.enter_context(tc.tile_pool(name="consts", bufs=1))
        spool = ctx.enter_context(tc.tile_pool(name="seq", bufs=2))
        y2pool = ctx.enter_context(tc.tile_pool(name="y2", bufs=1))
        vpool = ctx.enter_context(tc.tile_pool(name="vsb", bufs=1))
        epool = ctx.enter_context(tc.tile_pool(name="eexp", bufs=6))
        opool = ctx.enter_context(tc.tile_pool(name="otp", bufs=2))
        tpool = ctx.enter_context(tc.tile_pool(name="tmp", bufs=2))
        rpool = ctx.enter_context(tc.tile_pool(name="rr", bufs=2))
        spsum = ctx.enter_context(tc.tile_pool(name="S", bufs=2, space="PSUM"))
        qpool = ctx.enter_context(tc.tile_pool(name="qp", bufs=2, space="PSUM"))
        ypool = ctx.enter_context(tc.tile_pool(name="yp", bufs=2, space="PSUM"))

        def load_const(nm, dram_ap, shape, dt=F32, eng=None):
            t = cpool.tile(shape, dt, name=nm, tag=nm)
            (eng or nc.sync).dma_start(t[:], dram_ap)
            return t

        # ---- input DMAs spread across 4 engine queues: each DMA_DIRECT2D
        # issue costs ~1us of sequencer time, so a single queue serializes
        # the transfer STARTS (fill was issue-bound at 249GB/s). sync owns
        # the critical x stream; scalar/vector/gpsimd take the rest. ----
        qw = {}
        xq = [
            cpool.tile([128, CT * 512], BF16, name=f"xq{q}", tag=f"xq{q}")
            for q in range(4)
        ]
        qw[(0, 0)] = load_const("w00", qkvwT_d[0, 0], [128, CT * 128], BF16)
        # x quarter q: [128, CT*512], k-th tile's 512 tokens at cols 512k.
        # Quarter 0 lands in two halves so the first QKV group starts sooner.
        nc.sync.dma_start(xq[0][:, 0 : 4 * 512], xT_d[0, :, 0:4, :])
        qw[(0, 1)] = load_const("w01", qkvwT_d[0, 1], [128, CT * 128], BF16, eng=nc.scalar)
        cosT = cpool.tile([128, T], BF16, name="c_cos", tag="c_cos")
        sinTt = cpool.tile([128, T], BF16, name="c_sin", tag="c_sin")
        nc.scalar.dma_start(cosT[:, 0:512], cosT_d[:, 0:512])
        nc.scalar.dma_start(sinTt[:, 0:512], sinTt_d[:, 0:512])
        nc.sync.dma_start(xq[0][:, 4 * 512 :], xT_d[0, :, 4:CT, :])
        qw[(0, 2)] = load_const("w02", qkvwT_d[0, 2], [128, CT * 128], BF16, eng=nc.scalar)
        triC = load_const("c_tri", triC_d[:, :], [128, 128], BF16, eng=nc.gpsimd)
        expb = load_const("c_eb", expb_d[:, :], [128, NTK], eng=nc.gpsimd)
        nc.sync.dma_start(xq[1][:], xT_d[1])
        nc.sync.dma_start(xq[2][:], xT_d[2])
        nc.sync.dma_start(xq[3][:], xT_d[3])
        nc.scalar.dma_start(cosT[:, 512:T], cosT_d[:, 512:T])
        nc.scalar.dma_start(sinTt[:, 512:T], sinTt_d[:, 512:T])
        ow = []
        if has_qkv_bias:
            # tiny; loaded early because quarter 0 of QKV(0) needs hp0's
            bqkv = [
                load_const(f"c_bq{hp}", bqkv_d[hp], [128, 4], eng=nc.scalar)
                for hp in range(2)
            ]
            vbb = [
                load_const(f"c_vb{hp}", vbb_d[hp], [128, 128], eng=nc.scalar)
                for hp in range(2)
            ]

        def load_late_weights():
            # deferred: hp1 weights + ow aren't needed until attn0 / attn1.
            # Emitted after QKV quarter 0 so their gpsimd DMA issues queue
            # behind the first RoPE adds (~6us) and the x stream keeps
            # exclusive HBM bandwidth through the fill.
            qw[(1, 0)] = load_const(
                "w10", qkvwT_d[1, 0], [128, CT * 128], BF16, eng=nc.gpsimd
            )
            qw[(1, 1)] = load_const(
                "w11", qkvwT_d[1, 1], [128, CT * 128], BF16, eng=nc.gpsimd
            )
            qw[(1, 2)] = load_const(
                "w12", qkvwT_d[1, 2], [128, CT * 128], BF16, eng=nc.gpsimd
            )
            ow.extend(
                load_const(f"ow{hp}", owT_d[hp], [128, 8 * 128], BF16, eng=nc.gpsimd)
                for hp in range(2)
            )

        # dummy exp so the ACT table set loads during the initial DMA fill
        warm = cpool.tile([128, 1], F32, name="warm", tag="warm")
        nc.vector.memset(warm[:], 0.0)
        nc.scalar.activation(warm[:], warm[:], EXP)

        # HAM pre-warming: the PE clock sits at 1.2GHz until ~3.4us of
        # sustained activity, and fill-phase DMA gaps (>3.4us) keep
        # resetting the window, so QKV used to run cold until ~17us.
        # Dummy accumulating matmuls on zero tiles at the known stall
        # points keep the duty cycle up so real work runs at 2.4GHz.
        dumw = cpool.tile([128, 128], BF16, name="dumw", tag="dumw")
        dumx = cpool.tile([128, 512], BF16, name="dumx", tag="dumx")
        nc.vector.memset(dumw[:], 0.0)
        nc.vector.memset(dumx[:], 0.0)
        _warm_ctr = [0]

        def pe_warm(n):
            _warm_ctr[0] += 1
            dp = qpool.tile(
                [128, 512], F32, tag="qp", name=f"wmm{_warm_ctr[0]}"
            )
            for i in range(n):
                nc.tensor.matmul(
                    dp[:], dumw[:], dumx[:], start=(i == 0), stop=(i == n - 1)
                )

        # persistent AV stationary tiles [V_A | ones | V_B | ones]; the ones
        # halves (written once) make the AV matmul emit softmax row sums
        onesrc = cpool.tile([128, 64], F32, name="onesrc", tag="onesrc")
        nc.vector.memset(onesrc[:], 1.0)
        vsb = {}
        for hp in range(2):
            for t in range(NTK):
                vs = vpool.tile([128, 256], BF16, tag=f"vs{hp}_{t}", name=f"vs{hp}_{t}")
                nc.vector.tensor_copy(vs[:, 64:128], onesrc[:])
                nc.vector.tensor_copy(vs[:, 192:256], onesrc[:])
                vsb[(hp, t)] = vs

        y2T = {}
        qk2T = {}

        def qkv_stage(hp):
            q2T = spool.tile([128, T], BF16, tag="q2T", name=f"q2T{hp}")
            k2T = spool.tile([128, T], BF16, tag="k2T", name=f"k2T{hp}")
            qk2T[hp] = (q2T, k2T)
            dsts = [q2T, k2T]
            for jc in range(NJ):
                sl = slice(512 * jc, 512 * (jc + 1))
                for s in range(2):
                    ps = qpool.tile([128, 512], F32, tag="qp", name=f"ps{hp}_{jc}_{s}")
                    for k in range(CT):
                        nc.tensor.matmul(
                            ps[:],
                            qw[(hp, s)][:, 128 * k : 128 * (k + 1)],
                            xq[jc][:, 512 * k : 512 * (k + 1)],
                            start=(k == 0),
                            stop=(k == CT - 1),
                        )
                    if has_qkv_bias:
                        nc.vector.tensor_scalar_add(
                            ps[:], ps[:], bqkv[hp][:, s : s + 1]
                        )
                    t1 = tpool.tile([128, 512], BF16, tag="t1", name=f"t1_{hp}_{jc}_{s}")
                    t2 = tpool.tile([128, 512], BF16, tag="t2", name=f"t2_{hp}_{jc}_{s}")
                    t2s = tpool.tile(
                        [128, 512], BF16, tag="t2s", name=f"t2s_{hp}_{jc}_{s}"
                    )
                    nc.vector.tensor_mul(t1[:], ps[:], cosT[:, sl])
                    nc.vector.tensor_mul(t2[:], ps[:], sinTt[:, sl])
                    nc.vector.stream_shuffle(t2s[:], t2[:], SWAP_MASK)
                    nc.gpsimd.tensor_add(dsts[s][:, sl], t1[:], t2s[:])
                    yield
                # V token-major: 4 tiles of [128 tok, 128 dim] per chunk
                vt = qpool.tile([128, 512], F32, tag="qp", name=f"vt{hp}_{jc}")
                for u in range(4):
                    for k in range(CT):
                        nc.tensor.matmul(
                            vt[:, 128 * u : 128 * (u + 1)],
                            xq[jc][:, 512 * k + 128 * u : 512 * k + 128 * (u + 1)],
                            qw[(hp, 2)][:, 128 * k : 128 * (k + 1)],
                            start=(k == 0),
                            stop=(k == CT - 1),
                        )
                for u in range(4):
                    tki = 4 * jc + u
                    vs = vsb[(hp, tki)]
                    vtu = vt[:, 128 * u : 128 * (u + 1)]
                    vdst = vs[:, 0:192].rearrange("p (a b) -> p a b", b=64)[
                        :, 0::2, :
                    ]
                    if has_qkv_bias:
                        nc.vector.tensor_add(vdst, vtu, vbb[hp][:])
                    else:
                        nc.vector.tensor_copy(vdst, vtu)
                yield

        def out_proj_units(j):
            """Output projection for query block j: one unit per mt slice
            (2 accumulating matmuls + bf16 staging copy, alternating DVE/ACT),
            plus per-2mt SWDGE DMAs so the final drain is short."""
            jsl = slice(512 * j, 512 * (j + 1))
            otj = opool.tile([128, 8 * 512], BF16, tag="ot", name=f"ot{j}")
            for mt in range(8):
                def unit(mt=mt):
                    op = qpool.tile([128, 512], F32, tag="qp", name=f"op{j}_{mt}")
                    nc.tensor.matmul(
                        op[:], ow[0][:, 128 * mt : 128 * (mt + 1)],
                        y2T[0][:, jsl], start=True, stop=False,
                    )
                    nc.tensor.matmul(
                        op[:], ow[1][:, 128 * mt : 128 * (mt + 1)],
                        y2T[1][:, jsl], start=False, stop=True,
                    )
                    osl = slice(512 * mt, 512 * (mt + 1))
                    # all otj copies on DVE: attn(1)'s ACT is carrying exp;
                    # measured best balance (ACT variants re-throttle the PE)
                    nc.vector.tensor_copy(otj[:, osl], op[:])
                    # last block: per-mt DMAs (128KB) so the end-of-kernel
                    # drain never waits on a large in-flight transfer
                    if j == 3:
                        nc.gpsimd.dma_start(outp_d[:, mt, jsl], otj[:, osl])
                    elif mt % 2 == 1:
                        nc.gpsimd.dma_start(
                            outp_d[:, mt - 1 : mt + 1, jsl],
                            otj[:, 512 * (mt - 1) : 512 * (mt + 1)],
                        )
                yield unit

        def attention_stage(hp, jorder, feed, feed_rate, do_out, gates=None):
            # feed_rate may be fractional: consume floor increments of a
            # running budget so units spread evenly across the stage
            """Software-pipelined attention: AV(t) is emitted one tile late so
            S(t+1) sits ahead of it in the in-order PE queue while exp(t)
            runs. `feed` is a deque of callables (hp1 QKV units / out-proj
            units) drained between tiles to fill PE gaps. `gates[j]` = min
            feed units that must be consumed before block j is emitted (used
            to interleave QKV(0) quarters: block j needs k2T/V quarters <=j)."""
            q2T, k2T = qk2T[hp]
            pending = [None]
            budget = [0.0]
            consumed = [0]

            def drain_one():
                feed.popleft()()
                consumed[0] += 1

            def flush():
                if pending[0] is not None:
                    av, posts = pending[0]
                    pending[0] = None
                    av()
                    for p in posts:
                        p()

            for jidx, j in enumerate(jorder):
                rate = feed_rate[j] if isinstance(feed_rate, dict) else feed_rate
                if gates and j in gates:
                    gates[j]()  # pull prerequisite QKV quarters
                jsl = slice(512 * j, 512 * (j + 1))
                yp = [
                    ypool.tile([128, 512], F32, tag="yp", name=f"yp{hp}_{j}_{h}")
                    for h in range(2)
                ]
                ntk_j = 4 * (j + 1)
                for t in range(ntk_j):
                    tsl = slice(128 * t, 128 * (t + 1))
                    m = t - 4 * j if t >= 4 * j else -1
                    # diagonal tile m: query cols [0, 128m) see no valid keys
                    # in this tile; restrict S/exp/AV to cols [128m, 512).
                    ms = 128 * m if m >= 1 else 0
                    S = spsum.tile([128, 1024], F32, tag="S")
                    for h in range(2):
                        hsl = slice(64 * h, 64 * (h + 1))
                        nc.tensor.matmul(
                            S[:, 512 * h + ms : 512 * (h + 1)],
                            k2T[hsl, tsl],
                            q2T[hsl, 512 * j + ms : 512 * (j + 1)],
                            start=True,
                            stop=True,
                            tile_position=(64 * h, 0),
                        )
                    E = epool.tile([128, 1024], BF16, tag="E")
                    if m >= 1:
                        seg = E[:, 0:1024].rearrange("p (h c) -> p h c", h=2)[
                            :, :, 128 * m : 512
                        ]
                        sseg = S[:, 0:1024].rearrange("p (h c) -> p h c", h=2)[
                            :, :, 128 * m : 512
                        ]
                        nc.scalar.activation(
                            seg, sseg, EXP, bias=expb[:, t : t + 1], scale=0.125
                        )
                    else:
                        nc.scalar.activation(
                            E[:], S[:], EXP, bias=expb[:, t : t + 1], scale=0.125
                        )
                    if m >= 0:
                        for h in range(2):
                            nc.vector.tensor_mul(
                                E[:, 512 * h + 128 * m : 512 * h + 128 * (m + 1)],
                                E[:, 512 * h + 128 * m : 512 * h + 128 * (m + 1)],
                                triC[:, 0:128],
                            )
                    flush()

                    def mk_av(j=j, t=t, m=m, E=E, yp=yp, last=(t == ntk_j - 1)):
                        ma = 128 * m if m >= 1 else 0
                        for h in range(2):
                            nc.tensor.matmul(
                                yp[h][:, ma:512],
                                vsb[(hp, t)][:, 128 * h : 128 * (h + 1)],
                                E[:, 512 * h + ma : 512 * (h + 1)],
                                start=(t == 0),
                                stop=last,
                                skip_group_check=True,
                            )

                    posts = []
                    if t == ntk_j - 1:

                        def normalize(j=j, jsl=jsl, yp=yp):
                            # 1/den via the 1-instruction approx reciprocal
                            # (~51 ULP, ~5x faster than nc.vector.reciprocal,
                            # whose 4us latency stalled the PE: yp PSUM bufs
                            # can't recycle until normalize reads them).
                            # den copies go to ACT so the copy(h1) overlaps
                            # recip(h0) on DVE instead of serializing 6 ops.
                            dens = []
                            for h in range(2):
                                den = rpool.tile(
                                    [64, 512], F32, tag="dn", name=f"dn{hp}_{j}_{h}"
                                )
                                # stage PSUM->SBUF first: the BITWISE_NOT seed
                                # of the approx reciprocal misreads PSUM's
                                # accumulator format (0.39 rel err direct).
                                # hp0 boundaries land in attn0 (ACT slack);
                                # hp1's in attn1 where ACT carries exp+otj.
                                if hp == 0:
                                    nc.scalar.copy(den[:], yp[h][64:128, :])
                                else:
                                    nc.vector.tensor_copy(den[:], yp[h][64:128, :])
                                dens.append(den)
                            for h in range(2):
                                hsl = slice(64 * h, 64 * (h + 1))
                                rr = rpool.tile(
                                    [64, 512], F32, tag="rr", name=f"rr{hp}_{j}_{h}"
                                )
                                nc.vector.reciprocal_approx_fast(
                                    rr[:], dens[h][:]
                                )
                                nc.vector.tensor_mul(
                                    y2T[hp][hsl, jsl], yp[h][0:64, :], rr[:]
                                )
                            if do_out:
                                feed.extend(out_proj_units(j))

                        posts.append(normalize)
                    pending[0] = (mk_av, posts)
                    budget[0] += rate
                    while budget[0] >= 1.0 and feed:
                        budget[0] -= 1.0
                        drain_one()
                    yield
            flush()
            while feed:
                drain_one()

        # ---- schedule: QKV(0) quarter 0 only; attn(0) starts immediately
        # with the remaining QKV(0) quarters + all of QKV(1) as feed (gated
        # so quarter j lands before attention block j). This overlaps the
        # serial x-DMA fill with attn exp instead of stalling the PE. ----
        from collections import deque

        pe_warm(6)  # covers the ~2.5us wait for w00+xq0 before quarter 0
        g0 = qkv_stage(0)
        for _ in range(3):  # quarter 0: s=0, s=1, V
            next(g0)
        load_late_weights()
        g1 = qkv_stage(1)
        cnt0, cnt1 = [3], [0]

        def qkv0_unit():
            next(g0, None)
            cnt0[0] += 1

        def qkv1_unit():
            next(g1, None)
            cnt1[0] += 1

        def drain0(target):
            while cnt0[0] < target:
                qkv0_unit()

        def drain1(target):
            while cnt1[0] < target:
                qkv1_unit()

        feed0 = deque([qkv0_unit] * 9 + [qkv1_unit] * 6)
        y2T[0] = y2pool.tile([128, T], BF16, tag="y2T0", name="y2T0")
        # gates pull QKV quarters <=j before attention block j; rate 0 in
        # block 0: feed units would stall the in-order PE queue on the xq1
        # DMA and block the attention matmuls queued behind them
        # each gate leads with warmers sized to the typical xq DMA wait so
        # the PE queue stays active (and the clock warm) through the stall
        def gate0(n_warm, target):
            pe_warm(n_warm)
            drain0(target)

        gates0 = {
            1: lambda: gate0(6, 6),
            2: lambda: gate0(4, 9),
            3: lambda: gate0(3, 12),
        }
        rates0 = {0: 0.0, 1: 0.75, 2: 0.75, 3: 0.75}
        for _ in attention_stage(
            0, [0, 1, 2, 3], feed0, rates0, do_out=False, gates=gates0
        ):
            pass
        while feed0:
            feed0.popleft()()
        drain0(12)
        y2T[1] = y2pool.tile([128, T], BF16, tag="y2T1", name="y2T1")
        # QKV(1) quarters 2-3 are held back as attn(1)'s initial feed: attn1
        # starts with no OP units (first normalize hasn't run), so without
        # this the PE starves behind ACT exp for the first ~20 tiles.
        feed1 = deque([qkv1_unit] * 7)
        gates1 = {
            0: lambda: drain1(3),
            1: lambda: drain1(6),
            2: lambda: drain1(9),
            3: lambda: drain1(12),
        }
        rates1 = {0: 0.8, 1: 1.5, 2: 0.8, 3: 0.55}
        for _ in attention_stage(
            1, [0, 1, 2, 3], feed1, rates1, do_out=True, gates=gates1
        ):
            pass

    nc.compile()
    return nc


def _round_fp32r(a):
    """Round-to-nearest-even to fp32r (1s+8e+11m, value kept in the fp32 high bits)."""
    u = np.ascontiguousarray(a, np.float32).view(np.uint32)
    keep = u & np.uint32(0xFFFFF000)
    rem = u & np.uint32(0x00000FFF)
    lsb = (u >> np.uint32(12)) & np.uint32(1)
    up = (rem > 0x800) | ((rem == 0x800) & (lsb == 1))
    return (keep + (up.astype(np.uint32) << np.uint32(12))).view(np.float32)


def _host_inputs(x, attention_mask, qkv_w, qkv_b, out_w):
    """Build device input tensors. Returns per-core list of dicts."""
    import ml_dtypes

    BF = ml_dtypes.bfloat16
    x = np.ascontiguousarray(np.asarray(x, np.float32))
    qkv_w = np.asarray(qkv_w, np.float32)
    qkv_b = np.asarray(qkv_b, np.float32)
    out_w = np.asarray(out_w, np.float32)
    am = np.asarray(attention_mask)

    # xT[q, p, k, t'] = x[b][512q + t', 128k + p] (quarter-major so each
    # x-quarter DMA is one contiguous 1MB read)
    xT_b = [
        np.ascontiguousarray(
            x[b].T.reshape(CT, 128, NJ, 512).transpose(2, 1, 0, 3).astype(BF)
        )
        for b in range(B)
    ]

    # RoPE tables (match reference: interleaved rotate, concatenated freq table)
    inv_freq = 1.0 / (10000.0 ** (np.arange(0, DH, 2, dtype=np.float64) / DH))
    tt = np.arange(T, dtype=np.float64)
    freqs = np.outer(tt, inv_freq)  # [T, 32]
    emb = np.concatenate([freqs, freqs], axis=-1)  # [T, 64]
    cos = np.cos(emb).astype(np.float32).T  # [64, T]
    sin = np.sin(emb).astype(np.float32).T  # [64, T]
    sinTt64 = np.empty((DH, T), np.float32)
    sinTt64[0::2] = sin[1::2]  # sinTt[2i]   = +sin[2i+1]
    sinTt64[1::2] = -sin[0::2]  # sinTt[2i+1] = -sin[2i]
    cosT = np.ascontiguousarray(np.tile(cos, (2, 1)).astype(BF))  # [128, T]
    sinTt = np.ascontiguousarray(np.tile(sinTt64, (2, 1)).astype(BF))

    # tri mask for the diagonal partial block: query 512j+128m+c' vs key
    # 512j+128m+p -> valid iff c' >= p, identical for every m.
    cc = np.arange(128)[None, :]
    pp = np.arange(128)[:, None]
    triC = np.ascontiguousarray((cc >= pp).astype(BF))

    key_ok = am.astype(bool).reshape(B, NTK, 128)  # [b, t, p]
    expb_b = [
        np.ascontiguousarray(np.where(key_ok[b], 0.0, NEG).astype(np.float32).T)
        for b in range(B)
    ]

    per_core = []
    for c in range(NCORES):
        b_c, hg = divmod(c, 4)
        # qkvwT[hp, s, p, k, m] = qkv_w[s*C + r0 + m, 128k + p]
        qkvwT = np.empty((2, 3, 128, CT, 128), np.float32)
        bqkv = np.zeros((2, 128, 4), np.float32)
        vbb = np.empty((2, 128, 128), np.float32)
        owT = np.empty((2, 128, 8, 128), np.float32)
        for hp in range(2):
            r0 = 256 * hg + 128 * hp
            for s in range(3):
                w = qkv_w[s * C + r0 : s * C + r0 + 128, :]  # [rows 128, C]
                # -> [p, k, m]: w.T reshaped (CT, 128, C-part) transposed
                qkvwT[hp, s] = w.T.reshape(CT, 128, 128).transpose(1, 0, 2)
                if s < 2:
                    bqkv[hp, :, s] = qkv_b[s * C + r0 : s * C + r0 + 128]
            vbb[hp] = np.broadcast_to(
                qkv_b[2 * C + r0 : 2 * C + r0 + 128][None, :], (128, 128)
            )
            ow = out_w[:, r0 : r0 + 128]  # [1024, 128]
            # owT[p, mt, m] = out_w[128mt + m, r0 + p]
            owT[hp] = ow.reshape(8, 128, 128).transpose(2, 0, 1)
        per_core.append(
            dict(
                xT=xT_b[b_c],
                qkvwT=np.ascontiguousarray(qkvwT.astype(BF)),
                bqkv=bqkv,
                vbb=vbb,
                owT=np.ascontiguousarray(owT.astype(BF)),
                cosT=cosT,
                sinTt=sinTt,
                triC=triC,
                expb=expb_b[b_c],
            )
        )
    return per_core


def _gather(results, attention_mask, out_b):
    acc = np.zeros((B, T, C), np.float64)
    for c in range(NCORES):
        part = np.asarray(results[c]["outp"], np.float32)  # [128, 8, T]
        acc[c // 4] += part.transpose(1, 0, 2).reshape(C, T).T
    qm = np.asarray(attention_mask).astype(bool)
    out = np.where(qm[..., None], acc, 0.0) + np.asarray(out_b, np.float64)[None, None]
    return out.astype(np.float32)


def kernel(x, attention_mask, qkv_w, qkv_b, out_w, out_b, _trace=False):
    global LAST_RESULTS
    from concourse.bass_utils import run_bass_kernel_spmd

    key = ("nc", bool(np.any(np.asarray(qkv_b))))
    if key not in _PROGRAM_CACHE:
        _PROGRAM_CACHE[key] = _build_program(has_qkv_bias=key[1])
    nc = _PROGRAM_CACHE[key]

    in_maps = _host_inputs(x, attention_mask, qkv_w, qkv_b, out_w)

    res = run_bass_kernel_spmd(
        nc,
        in_maps,
        core_ids=list(range(NCORES)),
        trace=_trace,
        trace_cores=list(range(NCORES)) if _trace else None,
        stitch_traces=bool(_trace),
    )
    LAST_RESULTS = res
    return _gather(res.results, attention_mask, out_b)



# revision 56
# speedup vs baseline: 1.0306x; 1.0306x over previous
"""Causal self-attention with interleaved RoPE on 8 NeuronCores.

Sharding: batch x tensor-parallel. Core c handles batch c//4 and heads
4*(c%4) .. 4*(c%4)+3 (two head-pairs hp=0,1). Each core loads only its
batch's activations (bf16), computes QKV + RoPE + attention for its 4
heads, and writes a bf16 partial output [1024, T] (contraction over its
256 head dims); the host sums 4 partials per batch and adds the bias.

Per-core structure (per head-pair hp, packed heads hA, hB):
  - On-chip tensors live "transposed": feature dim on partitions, tokens
    on the free dim. Input DMAs are spread across the sync/scalar/gpsimd
    queues (each DMA_DIRECT2D issue costs ~1us of sequencer time); the x
    stream owns sync and late-needed weights are issued behind quarter-0
    compute so x keeps the HBM bandwidth through the fill.
  - QKV q,k: psum[row, tok] = w_tile.T @ x_tile (contraction over C in 8
    bf16 tiles). RoPE applied in-transposed layout via DVE stream_shuffle
    with a sign-folded, pair-reindexed sin table.
  - V is produced directly token-major: psum[tok, dim] = x_blk.T @ wv_tile
    (stationary = x block, moving = v weights); one copy lands it in the
    persistent AV stationary tile [V_A | V_B | ones] whose ones block
    makes the AV matmul also emit softmax row sums.
  - Scores transposed: S^T[tk, tq] = K^T.T @ Q^T per head, 2 heads packed
    via PE row tiling. Causal masking via subrange matmuls/exp on diagonal
    tiles plus a host tri mask for the partial block. exp on ACT (scale
    1/8 + key-mask bias folded in).
  - Softmax normalize uses the 1-instruction DVE approx reciprocal (staged
    through SBUF; its bit-trick seed misreads PSUM) so the yp PSUM buffers
    recycle in ~2us instead of 8us of exact-reciprocal latency.
  - Schedule: QKV(0) quarter 0; attn(0) starts immediately, fed by the
    remaining QKV(0) quarters + QKV(1) quarters 0-1 (gated per block);
    attn(1) is fed by QKV(1) quarters 2-3 first (its out-proj units only
    appear after the first normalize) and then per-block out-proj units,
    with the last block's rate lowered so leftover units keep the PE (and
    the HAM clock) warm through the final normalize window.
  - Out-projection: per query block, 8 units of 2 accumulating bf16
    matmuls (contraction 256 over both hps) + DVE bf16 copy; bf16 partials
    leave via gpsimd SWDGE in per-2mt pieces so the final drain never sits
    on a large transfer.
"""

import numpy as np

B, T, C = 2, 2048, 1024
H, DH = 16, 64
NCORES = 8
CT = C // 128  # 8 contraction tiles
NTK = T // 128  # 16 key tiles
NJ = T // 512  # 4 query blocks
NEG = -1e30

_PROGRAM_CACHE = {}
LAST_RESULTS = None


def _build_program(has_qkv_bias=False):
    import concourse.mybir as mybir
    import concourse.tile as tile
    from concourse import bacc
    from contextlib import ExitStack

    F32 = mybir.dt.float32
    F32R = mybir.dt.float32r
    BF16 = mybir.dt.bfloat16
    EXP = mybir.ActivationFunctionType.Exp
    LN = mybir.ActivationFunctionType.Ln

    SWAP_MASK = [i ^ 1 for i in range(32)]
    nc = bacc.Bacc("TRN2", target_bir_lowering=False, debug=False)

    # ---- DRAM I/O ----
    xT_d = nc.dram_tensor("xT", (NJ, 128, CT, 512), BF16, kind="ExternalInput")
    qkvwT_d = nc.dram_tensor("qkvwT", (2, 3, 128, CT, 128), BF16, kind="ExternalInput")
    bqkv_d = nc.dram_tensor("bqkv", (2, 128, 4), F32, kind="ExternalInput")
    vbb_d = nc.dram_tensor("vbb", (2, 128, 128), F32, kind="ExternalInput")
    owT_d = nc.dram_tensor("owT", (2, 128, 8, 128), BF16, kind="ExternalInput")
    cosT_d = nc.dram_tensor("cosT", (128, T), BF16, kind="ExternalInput")
    sinTt_d = nc.dram_tensor("sinTt", (128, T), BF16, kind="ExternalInput")
    triC_d = nc.dram_tensor("triC", (128, 128), BF16, kind="ExternalInput")
    expb_d = nc.dram_tensor("expb", (128, NTK), F32, kind="ExternalInput")
    outp_d = nc.dram_tensor("outp", (128, 8, T), BF16, kind="ExternalOutput")

    with tile.TileContext(nc) as tc, ExitStack() as ctx:
        cpool = ctx.enter_context(tc.tile_pool(name="consts", bufs=1))
        spool = ctx.enter_context(tc.tile_pool(name="seq", bufs=2))
        y2pool = ctx.enter_context(tc.tile_pool(name="y2", bufs=1))
        vpool = ctx.enter_context(tc.tile_pool(name="vsb", bufs=1))
        epool = ctx.enter_context(tc.tile_pool(name="eexp", bufs=6))
        opool = ctx.enter_context(tc.tile_pool(name="otp", bufs=2))
        tpool = ctx.enter_context(tc.tile_pool(name="tmp", bufs=2))
        rpool = ctx.enter_context(tc.tile_pool(name="rr", bufs=2))
        spsum = ctx.enter_context(tc.tile_pool(name="S", bufs=2, space="PSUM"))
        qpool = ctx.enter_context(tc.tile_pool(name="qp", bufs=2, space="PSUM"))
        ypool = ctx.enter_context(tc.tile_pool(name="yp", bufs=2, space="PSUM"))

        def load_const(nm, dram_ap, shape, dt=F32, eng=None):
            t = cpool.tile(shape, dt, name=nm, tag=nm)
            (eng or nc.sync).dma_start(t[:], dram_ap)
            return t

        # ---- input DMAs spread across 4 engine queues: each DMA_DIRECT2D
        # issue costs ~1us of sequencer time, so a single queue serializes
        # the transfer STARTS (fill was issue-bound at 249GB/s). sync owns
        # the critical x stream; scalar/vector/gpsimd take the rest. ----
        qw = {}
        xq = [
            cpool.tile([128, CT * 512], BF16, name=f"xq{q}", tag=f"xq{q}")
            for q in range(4)
        ]
        qw[(0, 0)] = load_const("w00", qkvwT_d[0, 0], [128, CT * 128], BF16)
        # x quarter q: [128, CT*512], k-th tile's 512 tokens at cols 512k.
        # Quarter 0 lands in two halves so the first QKV group starts sooner.
        nc.sync.dma_start(xq[0][:, 0 : 4 * 512], xT_d[0, :, 0:4, :])
        qw[(0, 1)] = load_const("w01", qkvwT_d[0, 1], [128, CT * 128], BF16, eng=nc.scalar)
        cosT = cpool.tile([128, T], BF16, name="c_cos", tag="c_cos")
        sinTt = cpool.tile([128, T], BF16, name="c_sin", tag="c_sin")
        nc.scalar.dma_start(cosT[:, 0:512], cosT_d[:, 0:512])
        nc.scalar.dma_start(sinTt[:, 0:512], sinTt_d[:, 0:512])
        nc.sync.dma_start(xq[0][:, 4 * 512 :], xT_d[0, :, 4:CT, :])
        qw[(0, 2)] = load_const("w02", qkvwT_d[0, 2], [128, CT * 128], BF16, eng=nc.scalar)
        triC = load_const("c_tri", triC_d[:, :], [128, 128], BF16, eng=nc.gpsimd)
        expb = load_const("c_eb", expb_d[:, :], [128, NTK], eng=nc.gpsimd)
        nc.sync.dma_start(xq[1][:], xT_d[1])
        nc.sync.dma_start(xq[2][:], xT_d[2])
        nc.sync.dma_start(xq[3][:], xT_d[3])
        nc.scalar.dma_start(cosT[:, 512:T], cosT_d[:, 512:T])
        nc.scalar.dma_start(sinTt[:, 512:T], sinTt_d[:, 512:T])
        ow = []
        if has_qkv_bias:
            # tiny; loaded early because quarter 0 of QKV(0) needs hp0's
            bqkv = [
                load_const(f"c_bq{hp}", bqkv_d[hp], [128, 4], eng=nc.scalar)
                for hp in range(2)
            ]
            vbb = [
                load_const(f"c_vb{hp}", vbb_d[hp], [128, 128], eng=nc.scalar)
                for hp in range(2)
            ]

        def load_late_weights():
            # deferred: hp1 weights + ow aren't needed until attn0 / attn1.
            # Emitted after QKV quarter 0 so their gpsimd DMA issues queue
            # behind the first RoPE adds (~6us) and the x stream keeps
            # exclusive HBM bandwidth through the fill.
            qw[(1, 0)] = load_const(
                "w10", qkvwT_d[1, 0], [128, CT * 128], BF16, eng=nc.gpsimd
            )
            qw[(1, 1)] = load_const(
                "w11", qkvwT_d[1, 1], [128, CT * 128], BF16, eng=nc.gpsimd
            )
            qw[(1, 2)] = load_const(
                "w12", qkvwT_d[1, 2], [128, CT * 128], BF16, eng=nc.gpsimd
            )
            ow.extend(
                load_const(f"ow{hp}", owT_d[hp], [128, 8 * 128], BF16, eng=nc.gpsimd)
                for hp in range(2)
            )

        # dummy exp so the ACT table set loads during the initial DMA fill
        warm = cpool.tile([128, 1], F32, name="warm", tag="warm")
        nc.vector.memset(warm[:], 0.0)
        nc.scalar.activation(warm[:], warm[:], EXP)

        # persistent AV stationary tiles [V_A | ones | V_B | ones]; the ones
        # halves (written once) make the AV matmul emit softmax row sums
        onesrc = cpool.tile([128, 64], F32, name="onesrc", tag="onesrc")
        nc.vector.memset(onesrc[:], 1.0)
        vsb = {}
        for hp in range(2):
            for t in range(NTK):
                vs = vpool.tile([128, 256], BF16, tag=f"vs{hp}_{t}", name=f"vs{hp}_{t}")
                nc.vector.tensor_copy(vs[:, 64:128], onesrc[:])
                nc.vector.tensor_copy(vs[:, 192:256], onesrc[:])
                vsb[(hp, t)] = vs

        y2T = {}
        qk2T = {}

        def qkv_stage(hp):
            q2T = spool.tile([128, T], BF16, tag="q2T", name=f"q2T{hp}")
            k2T = spool.tile([128, T], BF16, tag="k2T", name=f"k2T{hp}")
            qk2T[hp] = (q2T, k2T)
            dsts = [q2T, k2T]
            for jc in range(NJ):
                sl = slice(512 * jc, 512 * (jc + 1))
                for s in range(2):
                    ps = qpool.tile([128, 512], F32, tag="qp", name=f"ps{hp}_{jc}_{s}")
                    for k in range(CT):
                        nc.tensor.matmul(
                            ps[:],
                            qw[(hp, s)][:, 128 * k : 128 * (k + 1)],
                            xq[jc][:, 512 * k : 512 * (k + 1)],
                            start=(k == 0),
                            stop=(k == CT - 1),
                        )
                    if has_qkv_bias:
                        nc.vector.tensor_scalar_add(
                            ps[:], ps[:], bqkv[hp][:, s : s + 1]
                        )
                    t1 = tpool.tile([128, 512], BF16, tag="t1", name=f"t1_{hp}_{jc}_{s}")
                    t2 = tpool.tile([128, 512], BF16, tag="t2", name=f"t2_{hp}_{jc}_{s}")
                    t2s = tpool.tile(
                        [128, 512], BF16, tag="t2s", name=f"t2s_{hp}_{jc}_{s}"
                    )
                    nc.vector.tensor_mul(t1[:], ps[:], cosT[:, sl])
                    nc.vector.tensor_mul(t2[:], ps[:], sinTt[:, sl])
                    nc.vector.stream_shuffle(t2s[:], t2[:], SWAP_MASK)
                    nc.gpsimd.tensor_add(dsts[s][:, sl], t1[:], t2s[:])
                    yield
                # V token-major: 4 tiles of [128 tok, 128 dim] per chunk
                vt = qpool.tile([128, 512], F32, tag="qp", name=f"vt{hp}_{jc}")
                for u in range(4):
                    for k in range(CT):
                        nc.tensor.matmul(
                            vt[:, 128 * u : 128 * (u + 1)],
                            xq[jc][:, 512 * k + 128 * u : 512 * k + 128 * (u + 1)],
                            qw[(hp, 2)][:, 128 * k : 128 * (k + 1)],
                            start=(k == 0),
                            stop=(k == CT - 1),
                        )
                for u in range(4):
                    tki = 4 * jc + u
                    vs = vsb[(hp, tki)]
                    vtu = vt[:, 128 * u : 128 * (u + 1)]
                    vdst = vs[:, 0:192].rearrange("p (a b) -> p a b", b=64)[
                        :, 0::2, :
                    ]
                    if has_qkv_bias:
                        nc.vector.tensor_add(vdst, vtu, vbb[hp][:])
                    else:
                        nc.vector.tensor_copy(vdst, vtu)
                yield

        def out_proj_units(j):
            """Output projection for query block j: one unit per mt slice
            (2 accumulating matmuls + bf16 staging copy, alternating DVE/ACT),
            plus per-2mt SWDGE DMAs so the final drain is short."""
            jsl = slice(512 * j, 512 * (j + 1))
            otj = opool.tile([128, 8 * 512], BF16, tag="ot", name=f"ot{j}")
            for mt in range(8):
                def unit(mt=mt):
                    op = qpool.tile([128, 512], F32, tag="qp", name=f"op{j}_{mt}")
                    nc.tensor.matmul(
                        op[:], ow[0][:, 128 * mt : 128 * (mt + 1)],
                        y2T[0][:, jsl], start=True, stop=False,
                    )
                    nc.tensor.matmul(
                        op[:], ow[1][:, 128 * mt : 128 * (mt + 1)],
                        y2T[1][:, jsl], start=False, stop=True,
                    )
                    osl = slice(512 * mt, 512 * (mt + 1))
                    # all otj copies on DVE: attn(1)'s ACT is carrying exp;
                    # measured best balance (ACT variants re-throttle the PE)
                    nc.vector.tensor_copy(otj[:, osl], op[:])
                    # last block: per-mt DMAs (128KB) so the end-of-kernel
                    # drain never waits on a large in-flight transfer
                    if j == 3:
                        nc.gpsimd.dma_start(outp_d[:, mt, jsl], otj[:, osl])
                    elif mt % 2 == 1:
                        nc.gpsimd.dma_start(
                            outp_d[:, mt - 1 : mt + 1, jsl],
                            otj[:, 512 * (mt - 1) : 512 * (mt + 1)],
                        )
                yield unit

        def attention_stage(hp, jorder, feed, feed_rate, do_out, gates=None):
            # feed_rate may be fractional: consume floor increments of a
            # running budget so units spread evenly across the stage
            """Software-pipelined attention: AV(t) is emitted one tile late so
            S(t+1) sits ahead of it in the in-order PE queue while exp(t)
            runs. `feed` is a deque of callables (hp1 QKV units / out-proj
            units) drained between tiles to fill PE gaps. `gates[j]` = min
            feed units that must be consumed before block j is emitted (used
            to interleave QKV(0) quarters: block j needs k2T/V quarters <=j)."""
            q2T, k2T = qk2T[hp]
            pending = [None]
            budget = [0.0]
            consumed = [0]

            def drain_one():
                feed.popleft()()
                consumed[0] += 1

            def flush():
                if pending[0] is not None:
                    av, posts = pending[0]
                    pending[0] = None
                    av()
                    for p in posts:
                        p()

            for jidx, j in enumerate(jorder):
                rate = feed_rate[j] if isinstance(feed_rate, dict) else feed_rate
                if gates and j in gates:
                    gates[j]()  # pull prerequisite QKV quarters
                jsl = slice(512 * j, 512 * (j + 1))
                yp = [
                    ypool.tile([128, 512], F32, tag="yp", name=f"yp{hp}_{j}_{h}")
                    for h in range(2)
                ]
                ntk_j = 4 * (j + 1)
                for t in range(ntk_j):
                    tsl = slice(128 * t, 128 * (t + 1))
                    m = t - 4 * j if t >= 4 * j else -1
                    # diagonal tile m: query cols [0, 128m) see no valid keys
                    # in this tile; restrict S/exp/AV to cols [128m, 512).
                    ms = 128 * m if m >= 1 else 0
                    S = spsum.tile([128, 1024], F32, tag="S")
                    for h in range(2):
                        hsl = slice(64 * h, 64 * (h + 1))
                        nc.tensor.matmul(
                            S[:, 512 * h + ms : 512 * (h + 1)],
                            k2T[hsl, tsl],
                            q2T[hsl, 512 * j + ms : 512 * (j + 1)],
                            start=True,
                            stop=True,
                            tile_position=(64 * h, 0),
                        )
                    E = epool.tile([128, 1024], BF16, tag="E")
                    if m >= 1:
                        seg = E[:, 0:1024].rearrange("p (h c) -> p h c", h=2)[
                            :, :, 128 * m : 512
                        ]
                        sseg = S[:, 0:1024].rearrange("p (h c) -> p h c", h=2)[
                            :, :, 128 * m : 512
                        ]
                        nc.scalar.activation(
                            seg, sseg, EXP, bias=expb[:, t : t + 1], scale=0.125
                        )
                    else:
                        nc.scalar.activation(
                            E[:], S[:], EXP, bias=expb[:, t : t + 1], scale=0.125
                        )
                    if m >= 0:
                        for h in range(2):
                            nc.vector.tensor_mul(
                                E[:, 512 * h + 128 * m : 512 * h + 128 * (m + 1)],
                                E[:, 512 * h + 128 * m : 512 * h + 128 * (m + 1)],
                                triC[:, 0:128],
                            )
                    flush()

                    def mk_av(j=j, t=t, m=m, E=E, yp=yp, last=(t == ntk_j - 1)):
                        ma = 128 * m if m >= 1 else 0
                        for h in range(2):
                            nc.tensor.matmul(
                                yp[h][:, ma:512],
                                vsb[(hp, t)][:, 128 * h : 128 * (h + 1)],
                                E[:, 512 * h + ma : 512 * (h + 1)],
                                start=(t == 0),
                                stop=last,
                                skip_group_check=True,
                            )

                    posts = []
                    if t == ntk_j - 1:

                        def normalize(j=j, jsl=jsl, yp=yp):
                            # 1/den via the 1-instruction approx reciprocal
                            # (~51 ULP, ~5x faster than nc.vector.reciprocal,
                            # whose 4us latency stalled the PE: yp PSUM bufs
                            # can't recycle until normalize reads them).
                            # den copies go to ACT so the copy(h1) overlaps
                            # recip(h0) on DVE instead of serializing 6 ops.
                            dens = []
                            for h in range(2):
                                den = rpool.tile(
                                    [64, 512], F32, tag="dn", name=f"dn{hp}_{j}_{h}"
                                )
                                # stage PSUM->SBUF first: the BITWISE_NOT seed
                                # of the approx reciprocal misreads PSUM's
                                # accumulator format (0.39 rel err direct).
                                # hp0 boundaries land in attn0 (ACT slack);
                                # hp1's in attn1 where ACT carries exp+otj.
                                if hp == 0:
                                    nc.scalar.copy(den[:], yp[h][64:128, :])
                                else:
                                    nc.vector.tensor_copy(den[:], yp[h][64:128, :])
                                dens.append(den)
                            for h in range(2):
                                hsl = slice(64 * h, 64 * (h + 1))
                                rr = rpool.tile(
                                    [64, 512], F32, tag="rr", name=f"rr{hp}_{j}_{h}"
                                )
                                nc.vector.reciprocal_approx_fast(
                                    rr[:], dens[h][:]
                                )
                                nc.vector.tensor_mul(
                                    y2T[hp][hsl, jsl], yp[h][0:64, :], rr[:]
                                )
                            if do_out:
                                feed.extend(out_proj_units(j))

                        posts.append(normalize)
                    pending[0] = (mk_av, posts)
                    budget[0] += rate
                    while budget[0] >= 1.0 and feed:
                        budget[0] -= 1.0
                        drain_one()
                    yield
            flush()
            while feed:
                drain_one()

        # ---- schedule: QKV(0) quarter 0 only; attn(0) starts immediately
        # with the remaining QKV(0) quarters + all of QKV(1) as feed (gated
        # so quarter j lands before attention block j). This overlaps the
        # serial x-DMA fill with attn exp instead of stalling the PE. ----
        from collections import deque

        g0 = qkv_stage(0)
        for _ in range(3):  # quarter 0: s=0, s=1, V
            next(g0)
        load_late_weights()
        g1 = qkv_stage(1)
        cnt0, cnt1 = [3], [0]

        def qkv0_unit():
            next(g0, None)
            cnt0[0] += 1

        def qkv1_unit():
            next(g1, None)
            cnt1[0] += 1

        def drain0(target):
            while cnt0[0] < target:
                qkv0_unit()

        def drain1(target):
            while cnt1[0] < target:
                qkv1_unit()

        feed0 = deque([qkv0_unit] * 9 + [qkv1_unit] * 6)
        y2T[0] = y2pool.tile([128, T], BF16, tag="y2T0", name="y2T0")
        # gates pull QKV quarters <=j before attention block j; rate 0 in
        # block 0: feed units would stall the in-order PE queue on the xq1
        # DMA and block the attention matmuls queued behind them
        gates0 = {1: lambda: drain0(6), 2: lambda: drain0(9), 3: lambda: drain0(12)}
        rates0 = {0: 0.0, 1: 0.75, 2: 0.75, 3: 0.75}
        for _ in attention_stage(
            0, [0, 1, 2, 3], feed0, rates0, do_out=False, gates=gates0
        ):
            pass
        while feed0:
            feed0.popleft()()
        drain0(12)
        y2T[1] = y2pool.tile([128, T], BF16, tag="y2T1", name="y2T1")
        # QKV(1) quarters 2-3 are held back as attn(1)'s initial feed: attn1
        # starts with no OP units (first normalize hasn't run), so without
        # this the PE starves behind ACT exp for the first ~20 tiles.
        feed1 = deque([qkv1_unit] * 7)
        gates1 = {
            0: lambda: drain1(3),
            1: lambda: drain1(6),
            2: lambda: drain1(9),
            3: lambda: drain1(12),
        }
        rates1 = {0: 0.8, 1: 1.5, 2: 0.8, 3: 0.55}
        for _ in attention_stage(
            1, [0, 1, 2, 3], feed1, rates1, do_out=True, gates=gates1
        ):
            pass

    nc.compile()
    return nc


def _round_fp32r(a):
    """Round-to-nearest-even to fp32r (1s+8e+11m, value kept in the fp32 high bits)."""
    u = np.ascontiguousarray(a, np.float32).view(np.uint32)
    keep = u & np.uint32(0xFFFFF000)
    rem = u & np.uint32(0x00000FFF)
    lsb = (u >> np.uint32(12)) & np.uint32(1)
    up = (rem > 0x800) | ((rem == 0x800) & (lsb == 1))
    return (keep + (up.astype(np.uint32) << np.uint32(12))).view(np.float32)


def _host_inputs(x, attention_mask, qkv_w, qkv_b, out_w):
    """Build device input tensors. Returns per-core list of dicts."""
    import ml_dtypes

    BF = ml_dtypes.bfloat16
    x = np.ascontiguousarray(np.asarray(x, np.float32))
    qkv_w = np.asarray(qkv_w, np.float32)
    qkv_b = np.asarray(qkv_b, np.float32)
    out_w = np.asarray(out_w, np.float32)
    am = np.asarray(attention_mask)

    # xT[q, p, k, t'] = x[b][512q + t', 128k + p] (quarter-major so each
    # x-quarter DMA is one contiguous 1MB read)
    xT_b = [
        np.ascontiguousarray(
            x[b].T.reshape(CT, 128, NJ, 512).transpose(2, 1, 0, 3).astype(BF)
        )
        for b in range(B)
    ]

    # RoPE tables (match reference: interleaved rotate, concatenated freq table)
    inv_freq = 1.0 / (10000.0 ** (np.arange(0, DH, 2, dtype=np.float64) / DH))
    tt = np.arange(T, dtype=np.float64)
    freqs = np.outer(tt, inv_freq)  # [T, 32]
    emb = np.concatenate([freqs, freqs], axis=-1)  # [T, 64]
    cos = np.cos(emb).astype(np.float32).T  # [64, T]
    sin = np.sin(emb).astype(np.float32).T  # [64, T]
    sinTt64 = np.empty((DH, T), np.float32)
    sinTt64[0::2] = sin[1::2]  # sinTt[2i]   = +sin[2i+1]
    sinTt64[1::2] = -sin[0::2]  # sinTt[2i+1] = -sin[2i]
    cosT = np.ascontiguousarray(np.tile(cos, (2, 1)).astype(BF))  # [128, T]
    sinTt = np.ascontiguousarray(np.tile(sinTt64, (2, 1)).astype(BF))

    # tri mask for the diagonal partial block: query 512j+128m+c' vs key
    # 512j+128m+p -> valid iff c' >= p, identical for every m.
    cc = np.arange(128)[None, :]
    pp = np.arange(128)[:, None]
    triC = np.ascontiguousarray((cc >= pp).astype(BF))

    key_ok = am.astype(bool).reshape(B, NTK, 128)  # [b, t, p]
    expb_b = [
        np.ascontiguousarray(np.where(key_ok[b], 0.0, NEG).astype(np.float32).T)
        for b in range(B)
    ]

    per_core = []
    for c in range(NCORES):
        b_c, hg = divmod(c, 4)
        # qkvwT[hp, s, p, k, m] = qkv_w[s*C + r0 + m, 128k + p]
        qkvwT = np.empty((2, 3, 128, CT, 128), np.float32)
        bqkv = np.zeros((2, 128, 4), np.float32)
        vbb = np.empty((2, 128, 128), np.float32)
        owT = np.empty((2, 128, 8, 128), np.float32)
        for hp in range(2):
            r0 = 256 * hg + 128 * hp
            for s in range(3):
                w = qkv_w[s * C + r0 : s * C + r0 + 128, :]  # [rows 128, C]
                # -> [p, k, m]: w.T reshaped (CT, 128, C-part) transposed
                qkvwT[hp, s] = w.T.reshape(CT, 128, 128).transpose(1, 0, 2)
                if s < 2:
                    bqkv[hp, :, s] = qkv_b[s * C + r0 : s * C + r0 + 128]
            vbb[hp] = np.broadcast_to(
                qkv_b[2 * C + r0 : 2 * C + r0 + 128][None, :], (128, 128)
            )
            ow = out_w[:, r0 : r0 + 128]  # [1024, 128]
            # owT[p, mt, m] = out_w[128mt + m, r0 + p]
            owT[hp] = ow.reshape(8, 128, 128).transpose(2, 0, 1)
        per_core.append(
            dict(
                xT=xT_b[b_c],
                qkvwT=np.ascontiguousarray(qkvwT.astype(BF)),
                bqkv=bqkv,
                vbb=vbb,
                owT=np.ascontiguousarray(owT.astype(BF)),
                cosT=cosT,
                sinTt=sinTt,
                triC=triC,
                expb=expb_b[b_c],
            )
        )
    return per_core


def _gather(results, attention_mask, out_b):
    acc = np.zeros((B, T, C), np.float64)
    for c in range(NCORES):
        part = np.asarray(results[c]["outp"], np.float32)  # [128, 8, T]
        acc[c // 4] += part.transpose(1, 0, 2).reshape(C, T).T
    qm = np.asarray(attention_mask).astype(bool)
    out = np.where(qm[..., None], acc, 0.0) + np.asarray(out_b, np.float64)[None, None]
    return out.astype(np.float32)


def kernel(x, attention_mask, qkv_w, qkv_b, out_w, out_b, _trace=False):
    global LAST_RESULTS
    from concourse.bass_utils import run_bass_kernel_spmd

    key = ("nc", bool(np.any(np.asarray(qkv_b))))
    if key not in _PROGRAM_CACHE:
        _PROGRAM_CACHE[key] = _build_program(has_qkv_bias=key[1])
    nc = _PROGRAM_CACHE[key]

    in_maps = _host_inputs(x, attention_mask, qkv_w, qkv_b, out_w)

    res = run_bass_kernel_spmd(
        nc,
        in_maps,
        core_ids=list(range(NCORES)),
        trace=_trace,
        trace_cores=list(range(NCORES)) if _trace else None,
        stitch_traces=bool(_trace),
    )
    LAST_RESULTS = res
    return _gather(res.results, attention_mask, out_b)



# revision 57
# speedup vs baseline: 1.0362x; 1.0054x over previous
"""Causal self-attention with interleaved RoPE on 8 NeuronCores.

Sharding: batch x tensor-parallel. Core c handles batch c//4 and heads
4*(c%4) .. 4*(c%4)+3 (two head-pairs hp=0,1). Each core loads only its
batch's activations (bf16), computes QKV + RoPE + attention for its 4
heads, and writes a bf16 partial output [1024, T] (contraction over its
256 head dims); the host sums 4 partials per batch and adds the bias.

Per-core structure (per head-pair hp, packed heads hA, hB):
  - On-chip tensors live "transposed": feature dim on partitions, tokens
    on the free dim. Input DMAs are spread across the sync/scalar/gpsimd
    queues (each DMA_DIRECT2D issue costs ~1us of sequencer time); the x
    stream owns sync and late-needed weights are issued behind quarter-0
    compute so x keeps the HBM bandwidth through the fill.
  - QKV q,k: psum[row, tok] = w_tile.T @ x_tile (contraction over C in 8
    bf16 tiles). RoPE applied in-transposed layout via DVE stream_shuffle
    with a sign-folded, pair-reindexed sin table.
  - V is produced directly token-major: psum[tok, dim] = x_blk.T @ wv_tile
    (stationary = x block, moving = v weights); one copy lands it in the
    persistent AV stationary tile [V_A | V_B | ones] whose ones block
    makes the AV matmul also emit softmax row sums.
  - Scores transposed: S^T[tk, tq] = K^T.T @ Q^T per head, 2 heads packed
    via PE row tiling. Causal masking via subrange matmuls/exp on diagonal
    tiles plus a host tri mask for the partial block. exp on ACT (scale
    1/8 + key-mask bias folded in).
  - Softmax normalize uses the 1-instruction DVE approx reciprocal (staged
    through SBUF; its bit-trick seed misreads PSUM) so the yp PSUM buffers
    recycle in ~2us instead of 8us of exact-reciprocal latency.
  - Schedule: QKV(0) quarter 0; attn(0) starts immediately, fed by the
    remaining QKV(0) quarters + QKV(1) quarters 0-1 (gated per block);
    attn(1) is fed by QKV(1) quarters 2-3 first (its out-proj units only
    appear after the first normalize) and then per-block out-proj units,
    with the last block's rate lowered so leftover units keep the PE (and
    the HAM clock) warm through the final normalize window.
  - Out-projection: per query block, 8 units of 2 accumulating bf16
    matmuls (contraction 256 over both hps) + DVE bf16 copy; bf16 partials
    leave via gpsimd SWDGE in per-2mt pieces so the final drain never sits
    on a large transfer.
"""

import numpy as np

B, T, C = 2, 2048, 1024
H, DH = 16, 64
NCORES = 8
CT = C // 128  # 8 contraction tiles
NTK = T // 128  # 16 key tiles
NJ = T // 512  # 4 query blocks
NEG = -1e30

_PROGRAM_CACHE = {}
LAST_RESULTS = None


def _build_program(has_qkv_bias=False):
    import concourse.mybir as mybir
    import concourse.tile as tile
    from concourse import bacc
    from contextlib import ExitStack

    F32 = mybir.dt.float32
    F32R = mybir.dt.float32r
    BF16 = mybir.dt.bfloat16
    EXP = mybir.ActivationFunctionType.Exp
    LN = mybir.ActivationFunctionType.Ln

    SWAP_MASK = [i ^ 1 for i in range(32)]
    nc = bacc.Bacc("TRN2", target_bir_lowering=False, debug=False)

    # ---- DRAM I/O ----
    xT_d = nc.dram_tensor("xT", (NJ, 128, CT, 512), BF16, kind="ExternalInput")
    qkvwT_d = nc.dram_tensor("qkvwT", (2, 3, 128, CT, 128), BF16, kind="ExternalInput")
    bqkv_d = nc.dram_tensor("bqkv", (2, 128, 4), F32, kind="ExternalInput")
    vbb_d = nc.dram_tensor("vbb", (2, 128, 128), F32, kind="ExternalInput")
    owT_d = nc.dram_tensor("owT", (2, 128, 8, 128), BF16, kind="ExternalInput")
    cosT_d = nc.dram_tensor("cosT", (128, T), BF16, kind="ExternalInput")
    sinTt_d = nc.dram_tensor("sinTt", (128, T), BF16, kind="ExternalInput")
    triC_d = nc.dram_tensor("triC", (128, 128), BF16, kind="ExternalInput")
    expb_d = nc.dram_tensor("expb", (128, NTK), F32, kind="ExternalInput")
    outp_d = nc.dram_tensor("outp", (128, 8, T), BF16, kind="ExternalOutput")

    with tile.TileContext(nc) as tc, ExitStack() as ctx:
        cpool = ctx.enter_context(tc.tile_pool(name="consts", bufs=1))
        spool = ctx.enter_context(tc.tile_pool(name="seq", bufs=2))
        y2pool = ctx.enter_context(tc.tile_pool(name="y2", bufs=1))
        vpool = ctx.enter_context(tc.tile_pool(name="vsb", bufs=1))
        epool = ctx.enter_context(tc.tile_pool(name="eexp", bufs=6))
        opool = ctx.enter_context(tc.tile_pool(name="otp", bufs=2))
        tpool = ctx.enter_context(tc.tile_pool(name="tmp", bufs=2))
        rpool = ctx.enter_context(tc.tile_pool(name="rr", bufs=2))
        spsum = ctx.enter_context(tc.tile_pool(name="S", bufs=2, space="PSUM"))
        qpool = ctx.enter_context(tc.tile_pool(name="qp", bufs=2, space="PSUM"))
        ypool = ctx.enter_context(tc.tile_pool(name="yp", bufs=2, space="PSUM"))

        def load_const(nm, dram_ap, shape, dt=F32, eng=None):
            t = cpool.tile(shape, dt, name=nm, tag=nm)
            (eng or nc.sync).dma_start(t[:], dram_ap)
            return t

        # ---- input DMAs spread across 4 engine queues: each DMA_DIRECT2D
        # issue costs ~1us of sequencer time, so a single queue serializes
        # the transfer STARTS (fill was issue-bound at 249GB/s). sync owns
        # the critical x stream; scalar/vector/gpsimd take the rest. ----
        qw = {}
        xq = [
            cpool.tile([128, CT * 512], BF16, name=f"xq{q}", tag=f"xq{q}")
            for q in range(4)
        ]
        qw[(0, 0)] = load_const("w00", qkvwT_d[0, 0], [128, CT * 128], BF16)
        # x quarter q: [128, CT*512], k-th tile's 512 tokens at cols 512k.
        # Quarter 0 lands in two halves so the first QKV group starts sooner.
        nc.sync.dma_start(xq[0][:, 0 : 4 * 512], xT_d[0, :, 0:4, :])
        qw[(0, 1)] = load_const("w01", qkvwT_d[0, 1], [128, CT * 128], BF16, eng=nc.scalar)
        cosT = cpool.tile([128, T], BF16, name="c_cos", tag="c_cos")
        sinTt = cpool.tile([128, T], BF16, name="c_sin", tag="c_sin")
        nc.scalar.dma_start(cosT[:, 0:512], cosT_d[:, 0:512])
        nc.scalar.dma_start(sinTt[:, 0:512], sinTt_d[:, 0:512])
        nc.sync.dma_start(xq[0][:, 4 * 512 :], xT_d[0, :, 4:CT, :])
        qw[(0, 2)] = load_const("w02", qkvwT_d[0, 2], [128, CT * 128], BF16, eng=nc.scalar)
        triC = load_const("c_tri", triC_d[:, :], [128, 128], BF16, eng=nc.gpsimd)
        expb = load_const("c_eb", expb_d[:, :], [128, NTK], eng=nc.gpsimd)
        nc.sync.dma_start(xq[1][:], xT_d[1])
        nc.sync.dma_start(xq[2][:], xT_d[2])
        nc.sync.dma_start(xq[3][:], xT_d[3])
        nc.scalar.dma_start(cosT[:, 512:T], cosT_d[:, 512:T])
        nc.scalar.dma_start(sinTt[:, 512:T], sinTt_d[:, 512:T])
        ow = []
        if has_qkv_bias:
            # tiny; loaded early because quarter 0 of QKV(0) needs hp0's
            bqkv = [
                load_const(f"c_bq{hp}", bqkv_d[hp], [128, 4], eng=nc.scalar)
                for hp in range(2)
            ]
            vbb = [
                load_const(f"c_vb{hp}", vbb_d[hp], [128, 128], eng=nc.scalar)
                for hp in range(2)
            ]

        def load_late_weights():
            # deferred: hp1 weights + ow aren't needed until attn0 / attn1.
            # Emitted after QKV quarter 0 so their gpsimd DMA issues queue
            # behind the first RoPE adds (~6us) and the x stream keeps
            # exclusive HBM bandwidth through the fill.
            qw[(1, 0)] = load_const(
                "w10", qkvwT_d[1, 0], [128, CT * 128], BF16, eng=nc.gpsimd
            )
            qw[(1, 1)] = load_const(
                "w11", qkvwT_d[1, 1], [128, CT * 128], BF16, eng=nc.gpsimd
            )
            qw[(1, 2)] = load_const(
                "w12", qkvwT_d[1, 2], [128, CT * 128], BF16, eng=nc.gpsimd
            )
            ow.extend(
                load_const(f"ow{hp}", owT_d[hp], [128, 8 * 128], BF16, eng=nc.gpsimd)
                for hp in range(2)
            )

        # dummy exp so the ACT table set loads during the initial DMA fill
        warm = cpool.tile([128, 1], F32, name="warm", tag="warm")
        nc.vector.memset(warm[:], 0.0)
        nc.scalar.activation(warm[:], warm[:], EXP)

        # persistent AV stationary tiles [V_A | ones | V_B | ones]; the ones
        # halves (written once) make the AV matmul emit softmax row sums
        onesrc = cpool.tile([128, 64], F32, name="onesrc", tag="onesrc")
        nc.vector.memset(onesrc[:], 1.0)
        vsb = {}
        for hp in range(2):
            for t in range(NTK):
                vs = vpool.tile([128, 256], BF16, tag=f"vs{hp}_{t}", name=f"vs{hp}_{t}")
                nc.vector.tensor_copy(vs[:, 64:128], onesrc[:])
                nc.vector.tensor_copy(vs[:, 192:256], onesrc[:])
                vsb[(hp, t)] = vs

        y2T = {}
        qk2T = {}

        def qkv_stage(hp):
            q2T = spool.tile([128, T], BF16, tag="q2T", name=f"q2T{hp}")
            k2T = spool.tile([128, T], BF16, tag="k2T", name=f"k2T{hp}")
            qk2T[hp] = (q2T, k2T)
            dsts = [q2T, k2T]
            for jc in range(NJ):
                sl = slice(512 * jc, 512 * (jc + 1))
                for s in range(2):
                    ps = qpool.tile([128, 512], F32, tag="qp", name=f"ps{hp}_{jc}_{s}")
                    for k in range(CT):
                        nc.tensor.matmul(
                            ps[:],
                            qw[(hp, s)][:, 128 * k : 128 * (k + 1)],
                            xq[jc][:, 512 * k : 512 * (k + 1)],
                            start=(k == 0),
                            stop=(k == CT - 1),
                        )
                    if has_qkv_bias:
                        nc.vector.tensor_scalar_add(
                            ps[:], ps[:], bqkv[hp][:, s : s + 1]
                        )
                    t1 = tpool.tile([128, 512], BF16, tag="t1", name=f"t1_{hp}_{jc}_{s}")
                    t2 = tpool.tile([128, 512], BF16, tag="t2", name=f"t2_{hp}_{jc}_{s}")
                    t2s = tpool.tile(
                        [128, 512], BF16, tag="t2s", name=f"t2s_{hp}_{jc}_{s}"
                    )
                    nc.vector.tensor_mul(t1[:], ps[:], cosT[:, sl])
                    nc.vector.tensor_mul(t2[:], ps[:], sinTt[:, sl])
                    nc.vector.stream_shuffle(t2s[:], t2[:], SWAP_MASK)
                    nc.gpsimd.tensor_add(dsts[s][:, sl], t1[:], t2s[:])
                    yield
                # V token-major: 4 tiles of [128 tok, 128 dim] per chunk
                vt = qpool.tile([128, 512], F32, tag="qp", name=f"vt{hp}_{jc}")
                for u in range(4):
                    for k in range(CT):
                        nc.tensor.matmul(
                            vt[:, 128 * u : 128 * (u + 1)],
                            xq[jc][:, 512 * k + 128 * u : 512 * k + 128 * (u + 1)],
                            qw[(hp, 2)][:, 128 * k : 128 * (k + 1)],
                            start=(k == 0),
                            stop=(k == CT - 1),
                        )
                for u in range(4):
                    tki = 4 * jc + u
                    vs = vsb[(hp, tki)]
                    vtu = vt[:, 128 * u : 128 * (u + 1)]
                    vdst = vs[:, 0:192].rearrange("p (a b) -> p a b", b=64)[
                        :, 0::2, :
                    ]
                    if has_qkv_bias:
                        nc.vector.tensor_add(vdst, vtu, vbb[hp][:])
                    else:
                        nc.vector.tensor_copy(vdst, vtu)
                yield

        def out_proj_units(j):
            """Output projection for query block j: one unit per mt slice
            (2 accumulating matmuls + bf16 staging copy, alternating DVE/ACT),
            plus per-2mt SWDGE DMAs so the final drain is short."""
            jsl = slice(512 * j, 512 * (j + 1))
            otj = opool.tile([128, 8 * 512], BF16, tag="ot", name=f"ot{j}")
            for mt in range(8):
                def unit(mt=mt):
                    op = qpool.tile([128, 512], F32, tag="qp", name=f"op{j}_{mt}")
                    nc.tensor.matmul(
                        op[:], ow[0][:, 128 * mt : 128 * (mt + 1)],
                        y2T[0][:, jsl], start=True, stop=False,
                    )
                    nc.tensor.matmul(
                        op[:], ow[1][:, 128 * mt : 128 * (mt + 1)],
                        y2T[1][:, jsl], start=False, stop=True,
                    )
                    osl = slice(512 * mt, 512 * (mt + 1))
                    # all otj copies on DVE: attn(1)'s ACT is carrying exp;
                    # measured best balance (ACT variants re-throttle the PE)
                    nc.vector.tensor_copy(otj[:, osl], op[:])
                    # last block: per-mt DMAs (128KB) so the end-of-kernel
                    # drain never waits on a large in-flight transfer.
                    # Issued on the sync engine (idle through attn1): HWDGE
                    # queue, and its exit drain is cheaper than SWDGE's.
                    if j == 3:
                        nc.sync.dma_start(outp_d[:, mt, jsl], otj[:, osl])
                    elif mt % 2 == 1:
                        nc.sync.dma_start(
                            outp_d[:, mt - 1 : mt + 1, jsl],
                            otj[:, 512 * (mt - 1) : 512 * (mt + 1)],
                        )
                yield unit

        def attention_stage(hp, jorder, feed, feed_rate, do_out, gates=None):
            # feed_rate may be fractional: consume floor increments of a
            # running budget so units spread evenly across the stage
            """Software-pipelined attention: AV(t) is emitted one tile late so
            S(t+1) sits ahead of it in the in-order PE queue while exp(t)
            runs. `feed` is a deque of callables (hp1 QKV units / out-proj
            units) drained between tiles to fill PE gaps. `gates[j]` = min
            feed units that must be consumed before block j is emitted (used
            to interleave QKV(0) quarters: block j needs k2T/V quarters <=j)."""
            q2T, k2T = qk2T[hp]
            pending = [None]
            budget = [0.0]
            consumed = [0]

            def drain_one():
                feed.popleft()()
                consumed[0] += 1

            def flush():
                if pending[0] is not None:
                    av, posts = pending[0]
                    pending[0] = None
                    av()
                    for p in posts:
                        p()

            for jidx, j in enumerate(jorder):
                rate = feed_rate[j] if isinstance(feed_rate, dict) else feed_rate
                if gates and j in gates:
                    gates[j]()  # pull prerequisite QKV quarters
                jsl = slice(512 * j, 512 * (j + 1))
                yp = [
                    ypool.tile([128, 512], F32, tag="yp", name=f"yp{hp}_{j}_{h}")
                    for h in range(2)
                ]
                ntk_j = 4 * (j + 1)
                for t in range(ntk_j):
                    tsl = slice(128 * t, 128 * (t + 1))
                    m = t - 4 * j if t >= 4 * j else -1
                    # diagonal tile m: query cols [0, 128m) see no valid keys
                    # in this tile; restrict S/exp/AV to cols [128m, 512).
                    ms = 128 * m if m >= 1 else 0
                    S = spsum.tile([128, 1024], F32, tag="S")
                    for h in range(2):
                        hsl = slice(64 * h, 64 * (h + 1))
                        nc.tensor.matmul(
                            S[:, 512 * h + ms : 512 * (h + 1)],
                            k2T[hsl, tsl],
                            q2T[hsl, 512 * j + ms : 512 * (j + 1)],
                            start=True,
                            stop=True,
                            tile_position=(64 * h, 0),
                        )
                    E = epool.tile([128, 1024], BF16, tag="E")
                    if m >= 1:
                        seg = E[:, 0:1024].rearrange("p (h c) -> p h c", h=2)[
                            :, :, 128 * m : 512
                        ]
                        sseg = S[:, 0:1024].rearrange("p (h c) -> p h c", h=2)[
                            :, :, 128 * m : 512
                        ]
                        nc.scalar.activation(
                            seg, sseg, EXP, bias=expb[:, t : t + 1], scale=0.125
                        )
                    else:
                        nc.scalar.activation(
                            E[:], S[:], EXP, bias=expb[:, t : t + 1], scale=0.125
                        )
                    if m >= 0:
                        for h in range(2):
                            nc.vector.tensor_mul(
                                E[:, 512 * h + 128 * m : 512 * h + 128 * (m + 1)],
                                E[:, 512 * h + 128 * m : 512 * h + 128 * (m + 1)],
                                triC[:, 0:128],
                            )
                    flush()

                    def mk_av(j=j, t=t, m=m, E=E, yp=yp, last=(t == ntk_j - 1)):
                        ma = 128 * m if m >= 1 else 0
                        for h in range(2):
                            nc.tensor.matmul(
                                yp[h][:, ma:512],
                                vsb[(hp, t)][:, 128 * h : 128 * (h + 1)],
                                E[:, 512 * h + ma : 512 * (h + 1)],
                                start=(t == 0),
                                stop=last,
                                skip_group_check=True,
                            )

                    posts = []
                    if t == ntk_j - 1:

                        def normalize(j=j, jsl=jsl, yp=yp):
                            # 1/den via the 1-instruction approx reciprocal
                            # (~51 ULP, ~5x faster than nc.vector.reciprocal,
                            # whose 4us latency stalled the PE: yp PSUM bufs
                            # can't recycle until normalize reads them).
                            # den copies go to ACT so the copy(h1) overlaps
                            # recip(h0) on DVE instead of serializing 6 ops.
                            dens = []
                            for h in range(2):
                                den = rpool.tile(
                                    [64, 512], F32, tag="dn", name=f"dn{hp}_{j}_{h}"
                                )
                                # stage PSUM->SBUF first: the BITWISE_NOT seed
                                # of the approx reciprocal misreads PSUM's
                                # accumulator format (0.39 rel err direct).
                                # hp0 boundaries land in attn0 (ACT slack);
                                # hp1's in attn1 where ACT carries exp+otj.
                                if hp == 0:
                                    nc.scalar.copy(den[:], yp[h][64:128, :])
                                else:
                                    nc.vector.tensor_copy(den[:], yp[h][64:128, :])
                                dens.append(den)
                            for h in range(2):
                                hsl = slice(64 * h, 64 * (h + 1))
                                rr = rpool.tile(
                                    [64, 512], F32, tag="rr", name=f"rr{hp}_{j}_{h}"
                                )
                                nc.vector.reciprocal_approx_fast(
                                    rr[:], dens[h][:]
                                )
                                nc.vector.tensor_mul(
                                    y2T[hp][hsl, jsl], yp[h][0:64, :], rr[:]
                                )
                            if do_out:
                                feed.extend(out_proj_units(j))

                        posts.append(normalize)
                    pending[0] = (mk_av, posts)
                    budget[0] += rate
                    while budget[0] >= 1.0 and feed:
                        budget[0] -= 1.0
                        drain_one()
                    yield
            flush()
            while feed:
                drain_one()

        # ---- schedule: QKV(0) quarter 0 only; attn(0) starts immediately
        # with the remaining QKV(0) quarters + all of QKV(1) as feed (gated
        # so quarter j lands before attention block j). This overlaps the
        # serial x-DMA fill with attn exp instead of stalling the PE. ----
        from collections import deque

        g0 = qkv_stage(0)
        for _ in range(3):  # quarter 0: s=0, s=1, V
            next(g0)
        load_late_weights()
        g1 = qkv_stage(1)
        cnt0, cnt1 = [3], [0]

        def qkv0_unit():
            next(g0, None)
            cnt0[0] += 1

        def qkv1_unit():
            next(g1, None)
            cnt1[0] += 1

        def drain0(target):
            while cnt0[0] < target:
                qkv0_unit()

        def drain1(target):
            while cnt1[0] < target:
                qkv1_unit()

        feed0 = deque([qkv0_unit] * 9 + [qkv1_unit] * 6)
        y2T[0] = y2pool.tile([128, T], BF16, tag="y2T0", name="y2T0")
        # gates pull QKV quarters <=j before attention block j; rate 0 in
        # block 0: feed units would stall the in-order PE queue on the xq1
        # DMA and block the attention matmuls queued behind them
        gates0 = {1: lambda: drain0(6), 2: lambda: drain0(9), 3: lambda: drain0(12)}
        rates0 = {0: 0.0, 1: 0.75, 2: 0.75, 3: 0.75}
        for _ in attention_stage(
            0, [0, 1, 2, 3], feed0, rates0, do_out=False, gates=gates0
        ):
            pass
        while feed0:
            feed0.popleft()()
        drain0(12)
        y2T[1] = y2pool.tile([128, T], BF16, tag="y2T1", name="y2T1")
        # QKV(1) quarters 2-3 are held back as attn(1)'s initial feed: attn1
        # starts with no OP units (first normalize hasn't run), so without
        # this the PE starves behind ACT exp for the first ~20 tiles.
        feed1 = deque([qkv1_unit] * 7)
        gates1 = {
            0: lambda: drain1(3),
            1: lambda: drain1(6),
            2: lambda: drain1(9),
            3: lambda: drain1(12),
        }
        rates1 = {0: 0.8, 1: 1.5, 2: 0.8, 3: 0.55}
        for _ in attention_stage(
            1, [0, 1, 2, 3], feed1, rates1, do_out=True, gates=gates1
        ):
            pass

    nc.compile()
    return nc


def _round_fp32r(a):
    """Round-to-nearest-even to fp32r (1s+8e+11m, value kept in the fp32 high bits)."""
    u = np.ascontiguousarray(a, np.float32).view(np.uint32)
    keep = u & np.uint32(0xFFFFF000)
    rem = u & np.uint32(0x00000FFF)
    lsb = (u >> np.uint32(12)) & np.uint32(1)
    up = (rem > 0x800) | ((rem == 0x800) & (lsb == 1))
    return (keep + (up.astype(np.uint32) << np.uint32(12))).view(np.float32)


def _host_inputs(x, attention_mask, qkv_w, qkv_b, out_w):
    """Build device input tensors. Returns per-core list of dicts."""
    import ml_dtypes

    BF = ml_dtypes.bfloat16
    x = np.ascontiguousarray(np.asarray(x, np.float32))
    qkv_w = np.asarray(qkv_w, np.float32)
    qkv_b = np.asarray(qkv_b, np.float32)
    out_w = np.asarray(out_w, np.float32)
    am = np.asarray(attention_mask)

    # xT[q, p, k, t'] = x[b][512q + t', 128k + p] (quarter-major so each
    # x-quarter DMA is one contiguous 1MB read)
    xT_b = [
        np.ascontiguousarray(
            x[b].T.reshape(CT, 128, NJ, 512).transpose(2, 1, 0, 3).astype(BF)
        )
        for b in range(B)
    ]

    # RoPE tables (match reference: interleaved rotate, concatenated freq table)
    inv_freq = 1.0 / (10000.0 ** (np.arange(0, DH, 2, dtype=np.float64) / DH))
    tt = np.arange(T, dtype=np.float64)
    freqs = np.outer(tt, inv_freq)  # [T, 32]
    emb = np.concatenate([freqs, freqs], axis=-1)  # [T, 64]
    cos = np.cos(emb).astype(np.float32).T  # [64, T]
    sin = np.sin(emb).astype(np.float32).T  # [64, T]
    sinTt64 = np.empty((DH, T), np.float32)
    sinTt64[0::2] = sin[1::2]  # sinTt[2i]   = +sin[2i+1]
    sinTt64[1::2] = -sin[0::2]  # sinTt[2i+1] = -sin[2i]
    cosT = np.ascontiguousarray(np.tile(cos, (2, 1)).astype(BF))  # [128, T]
    sinTt = np.ascontiguousarray(np.tile(sinTt64, (2, 1)).astype(BF))

    # tri mask for the diagonal partial block: query 512j+128m+c' vs key
    # 512j+128m+p -> valid iff c' >= p, identical for every m.
    cc = np.arange(128)[None, :]
    pp = np.arange(128)[:, None]
    triC = np.ascontiguousarray((cc >= pp).astype(BF))

    key_ok = am.astype(bool).reshape(B, NTK, 128)  # [b, t, p]
    expb_b = [
        np.ascontiguousarray(np.where(key_ok[b], 0.0, NEG).astype(np.float32).T)
        for b in range(B)
    ]

    per_core = []
    for c in range(NCORES):
        b_c, hg = divmod(c, 4)
        # qkvwT[hp, s, p, k, m] = qkv_w[s*C + r0 + m, 128k + p]
        qkvwT = np.empty((2, 3, 128, CT, 128), np.float32)
        bqkv = np.zeros((2, 128, 4), np.float32)
        vbb = np.empty((2, 128, 128), np.float32)
        owT = np.empty((2, 128, 8, 128), np.float32)
        for hp in range(2):
            r0 = 256 * hg + 128 * hp
            for s in range(3):
                w = qkv_w[s * C + r0 : s * C + r0 + 128, :]  # [rows 128, C]
                # -> [p, k, m]: w.T reshaped (CT, 128, C-part) transposed
                qkvwT[hp, s] = w.T.reshape(CT, 128, 128).transpose(1, 0, 2)
                if s < 2:
                    bqkv[hp, :, s] = qkv_b[s * C + r0 : s * C + r0 + 128]
            vbb[hp] = np.broadcast_to(
                qkv_b[2 * C + r0 : 2 * C + r0 + 128][None, :], (128, 128)
            )
            ow = out_w[:, r0 : r0 + 128]  # [1024, 128]
            # owT[p, mt, m] = out_w[128mt + m, r0 + p]
            owT[hp] = ow.reshape(8, 128, 128).transpose(2, 0, 1)
        per_core.append(
            dict(
                xT=xT_b[b_c],
                qkvwT=np.ascontiguousarray(qkvwT.astype(BF)),
                bqkv=bqkv,
                vbb=vbb,
                owT=np.ascontiguousarray(owT.astype(BF)),
                cosT=cosT,
                sinTt=sinTt,
                triC=triC,
                expb=expb_b[b_c],
            )
        )
    return per_core


def _gather(results, attention_mask, out_b):
    acc = np.zeros((B, T, C), np.float64)
    for c in range(NCORES):
        part = np.asarray(results[c]["outp"], np.float32)  # [128, 8, T]
        acc[c // 4] += part.transpose(1, 0, 2).reshape(C, T).T
    qm = np.asarray(attention_mask).astype(bool)
    out = np.where(qm[..., None], acc, 0.0) + np.asarray(out_b, np.float64)[None, None]
    return out.astype(np.float32)


def kernel(x, attention_mask, qkv_w, qkv_b, out_w, out_b, _trace=False):
    global LAST_RESULTS
    from concourse.bass_utils import run_bass_kernel_spmd

    key = ("nc", bool(np.any(np.asarray(qkv_b))))
    if key not in _PROGRAM_CACHE:
        _PROGRAM_CACHE[key] = _build_program(has_qkv_bias=key[1])
    nc = _PROGRAM_CACHE[key]

    in_maps = _host_inputs(x, attention_mask, qkv_w, qkv_b, out_w)

    res = run_bass_kernel_spmd(
        nc,
        in_maps,
        core_ids=list(range(NCORES)),
        trace=_trace,
        trace_cores=list(range(NCORES)) if _trace else None,
        stitch_traces=bool(_trace),
    )
    LAST_RESULTS = res
    return _gather(res.results, attention_mask, out_b)

